# revision 1
# baseline (speedup 1.0000x reference)
"""GCN layer (D^{-1/2} A D^{-1/2} X, aggregated to src rows, then Linear+ReLU)
as a Bass/Tile kernel on 8 Trainium2 NeuronCores.

Strategy:
  - Host: sort edges by src; core c owns src rows [c*6250, (c+1)*6250) and all
    edges whose src falls there. x is replicated to every core (no collectives).
    Per-edge norm = dinv[src]*dinv[dst] precomputed on host (cheap O(E) index math).
  - Device, per core: for each window of 128 src nodes, dma_gather the x[dst]
    rows (512B each) into SBUF; for each 128-edge block build a one-hot
    "selection" matrix with one DVE tensor_scalar op ((iota == src_rel) * norm),
    and accumulate aggT[feat, node] += y_block^T @ onehot on the TensorEngine
    in PSUM.  Epilogue per window: out = relu(agg @ W^T + b) via two matmuls
    (bias injected with a K=1 ones matmul) + one ScalarEngine Relu, then a
    contiguous DMA store.
  - dma_gather indices are int16, so x is addressed via two base regions
    (rows [0, 32768) and [32768, 50000)); each window issues one gather per
    region with per-window block counts fixed at trace time (max over the 8
    cores, so a single NEFF serves all cores SPMD).
"""

import ml_dtypes
import numpy as np

import concourse.bacc as bacc
import concourse.mybir as mybir
import concourse.tile as tile
from concourse.bass_utils import run_bass_kernel_spmd

N_NODES = 50000
N_EDGES = 800000
F = 128
N_CORES = 8
NODES_PER_CORE = N_NODES // N_CORES  # 6250
WIN = 128
N_WIN = -(-NODES_PER_CORE // WIN)  # 49
LO_BASE = 32768  # int16 index range per gather base
YBUFS = 3
OHBUFS = 6
PSABUFS = 2
PSOBUFS = 2
AGGBUFS = 3
OUTBUFS = 3
SINGLE_PACKET = False
NQ = 4
SCRATCH = 65536
GATHER_BF16 = True
SORT_DST = True
ONEHOT_WIDE = True
NORM_FOLD = True


def _pack_idx16(idxs: np.ndarray) -> np.ndarray:
    """Pack an index vector (len multiple of 16) into the dma_gather idx tile
    layout: element i -> [i % 16, i // 16], replicated over 8 partition groups."""
    n = len(idxs)
    p16 = idxs.reshape(n // 16, 16).T.astype(np.int16)  # [16, n//16]
    return np.tile(p16, (8, 1))  # [128, n//16]


def _host_prep(x, edge_index, W, b):
    src = np.asarray(edge_index[0], dtype=np.int64)
    dst = np.asarray(edge_index[1], dtype=np.int64)
    deg = np.bincount(src, minlength=N_NODES).astype(np.float32)
    dinv = np.where(deg > 0, 1.0 / np.sqrt(deg), 0.0).astype(np.float32)
    if NORM_FOLD:
        norm = np.ones(N_EDGES, dtype=np.float32)
    else:
        norm = (dinv[src] * dinv[dst]).astype(np.float32)

    order = np.argsort(src, kind="stable")
    src_s, dst_s, norm_s = src[order], dst[order], norm[order]

    # Split edges into (core, window, lo/hi) buckets.
    core_of = src_s // NODES_PER_CORE
    wloc = (src_s % NODES_PER_CORE) // WIN
    is_hi = dst_s >= LO_BASE

    # boundaries of each core's edge range in the sorted list
    core_starts = np.searchsorted(core_of, np.arange(N_CORES + 1))

    # per (core, window): edge index ranges; within a window, order lo first.
    buckets = {}
    n_lo = np.zeros((N_CORES, N_WIN), dtype=np.int64)
    n_hi = np.zeros((N_CORES, N_WIN), dtype=np.int64)
    for c in range(N_CORES):
        s, e = core_starts[c], core_starts[c + 1]
        wl = wloc[s:e]
        w_starts = np.searchsorted(wl, np.arange(N_WIN + 1)) + s
        for w in range(N_WIN):
            ws, we = w_starts[w], w_starts[w + 1]
            hi_m = is_hi[ws:we]
            lo_idx = np.arange(ws, we)[~hi_m]
            hi_idx = np.arange(ws, we)[hi_m]
            if SORT_DST:
                lo_idx = lo_idx[np.argsort(dst_s[lo_idx], kind="stable")]
                hi_idx = hi_idx[np.argsort(dst_s[hi_idx], kind="stable")]
            buckets[(c, w)] = (lo_idx, hi_idx)
            n_lo[c, w] = len(lo_idx)
            n_hi[c, w] = len(hi_idx)

    # Static per-window block counts (max over cores -> one NEFF for all).
    B_A = np.maximum(-(-n_lo.max(axis=0) // 128), 0).astype(np.int64)
    B_B = np.maximum(-(-n_hi.max(axis=0) // 128), 0).astype(np.int64)
    B_tot = B_A + B_B
    TB = int(B_tot.sum())  # total blocks per core
    Bmax = int(B_tot.max())

    # Pack per-core device inputs.
    idx16 = np.zeros((N_CORES, 128, TB * 8), dtype=np.int16)
    srel = np.full((N_CORES, 128, TB), 300.0, dtype=np.float32)  # 300 => no match
    normv = np.zeros((N_CORES, 128, TB), dtype=np.float32)

    for c in range(N_CORES):
        tb = 0
        col8 = 0
        for w in range(N_WIN):
            lo_idx, hi_idx = buckets[(c, w)]
            base_node = c * NODES_PER_CORE + w * WIN
            for edges, nblk, rebase in (
                (lo_idx, int(B_A[w]), 0),
                (hi_idx, int(B_B[w]), LO_BASE),
            ):
                if nblk == 0:
                    continue
                n = nblk * 128
                cnt = len(edges)
                dvals = np.zeros(n, dtype=np.int64)
                dvals[:cnt] = dst_s[edges] - rebase
                idx16[c, :, col8 : col8 + nblk * 8] = _pack_idx16(dvals)
                sv = np.full(n, 300.0, dtype=np.float32)
                sv[:cnt] = (src_s[edges] - base_node).astype(np.float32)
                nv = np.zeros(n, dtype=np.float32)
                nv[:cnt] = norm_s[edges]
                # edge i of this call -> (lane i%128, block tb + i//128)
                srel[c, :, tb : tb + nblk] = sv.reshape(nblk, 128).T
                normv[c, :, tb : tb + nblk] = nv.reshape(nblk, 128).T
                tb += nblk
                col8 += nblk * 8

    wt = np.ascontiguousarray(np.asarray(W, dtype=np.float32).T)  # [in, out]
    brow = np.asarray(b, dtype=np.float32).reshape(1, F)
    if ONEHOT_WIDE:
        iota = np.broadcast_to(
            np.arange(WIN, dtype=np.float32).astype(ml_dtypes.bfloat16), (128, Bmax, WIN)
        ).copy()
        srel = srel.astype(ml_dtypes.bfloat16)
        normv = normv.astype(ml_dtypes.bfloat16)
    else:
        iota = np.tile(np.arange(F, dtype=np.float32), (128, 1))

    prep_dinv_full = dinv.copy()
    dinv_col = np.zeros((N_CORES, WIN, N_WIN), dtype=np.float32)
    invd = np.zeros((N_CORES, 1, N_WIN * WIN), dtype=np.float32)
    for c in range(N_CORES):
        dv = np.zeros(N_WIN * WIN, dtype=np.float32)
        dv[:NODES_PER_CORE] = dinv[c * NODES_PER_CORE : (c + 1) * NODES_PER_CORE]
        dinv_col[c] = dv.reshape(N_WIN, WIN).T
        iv = np.zeros_like(dv)
        nz = dv > 0
        iv[nz] = 1.0 / dv[nz]
        invd[c, 0] = iv
    return {
        "deg": deg,
        "dinv_full": prep_dinv_full,
        "dinv_col": dinv_col,
        "invd": invd,
        "B_A": B_A,
        "B_B": B_B,
        "TB": TB,
        "Bmax": Bmax,
        "idx16": idx16,
        "srel": srel,
        "normv": normv,
        "wt": wt,
        "brow": brow,
        "iota": iota,
    }


def _build_program(B_A, B_B, TB, Bmax, repeat=1, mode="full"):
    f32 = mybir.dt.float32
    gdt = mybir.dt.bfloat16 if GATHER_BF16 else f32
    nc = bacc.Bacc(
        "TRN2",
        target_bir_lowering=False,
        debug=False,
        num_devices=1,
        num_swdge_queues=NQ,
        dynamic_dma_scratch_size=SCRATCH,
    )

    x_d = nc.dram_tensor("x", [N_NODES, F], gdt, kind="ExternalInput")
    idx_d = nc.dram_tensor("idx16", [128, TB * 8], mybir.dt.int16, kind="ExternalInput")
    mdt = gdt if ONEHOT_WIDE else f32
    srel_d = nc.dram_tensor("srel", [128, TB], mdt, kind="ExternalInput")
    normv_d = nc.dram_tensor("normv", [128, TB], mdt, kind="ExternalInput")
    wt_d = nc.dram_tensor("wt", [F, F], f32, kind="ExternalInput")
    brow_d = nc.dram_tensor("brow", [1, F], f32, kind="ExternalInput")
    dinv_d = nc.dram_tensor("dinvc", [WIN, N_WIN], f32, kind="ExternalInput")
    invd_d = nc.dram_tensor("invd", [1, N_WIN * WIN], f32, kind="ExternalInput")
    iota_shape = [128, Bmax, WIN] if ONEHOT_WIDE else [128, F]
    iota_d = nc.dram_tensor("iota", iota_shape, mdt if ONEHOT_WIDE else f32, kind="ExternalInput")
    out_d = nc.dram_tensor("out", [N_WIN, WIN, F], f32, kind="ExternalOutput")

    x_lo = x_d.ap()[0:LO_BASE, :]
    x_hi = x_d.ap()[LO_BASE:N_NODES, :]

    with tile.TileContext(nc) as tc:
        with (
            tc.tile_pool(name="const", bufs=1) as cpool,
            tc.tile_pool(name="y", bufs=YBUFS) as ypool,
            tc.tile_pool(name="oh", bufs=OHBUFS) as ohpool,
            tc.tile_pool(name="agg", bufs=AGGBUFS) as apool,
            tc.tile_pool(name="outp", bufs=OUTBUFS) as opool,
            tc.tile_pool(name="psA", bufs=PSABUFS, space="PSUM") as psA,
            tc.tile_pool(name="psO", bufs=PSOBUFS, space="PSUM") as psO,
        ):
            idx_sb = cpool.tile([128, TB * 8], mybir.dt.int16)
            nc.sync.dma_start(idx_sb[:], idx_d.ap())
            srel_sb = cpool.tile([128, TB], mdt)
            nc.sync.dma_start(srel_sb[:], srel_d.ap())
            normv_sb = cpool.tile([128, TB], mdt)
            nc.sync.dma_start(normv_sb[:], normv_d.ap())
            wt_sb = cpool.tile([F, F], f32)
            nc.sync.dma_start(wt_sb[:], wt_d.ap())
            brow_sb = cpool.tile([1, F], f32)
            nc.sync.dma_start(brow_sb[:], brow_d.ap())
            dinv_sb = cpool.tile([WIN, N_WIN], f32)
            nc.sync.dma_start(dinv_sb[:], dinv_d.ap())
            invd_sb = cpool.tile([1, N_WIN * WIN], f32)
            nc.sync.dma_start(invd_sb[:], invd_d.ap())
            iota_sb = cpool.tile(iota_shape, mdt if ONEHOT_WIDE else f32)
            nc.sync.dma_start(iota_sb[:], iota_d.ap())
            ones_sb = cpool.tile([1, F], f32)
            nc.vector.memset(ones_sb[:], 1.0)
            yt_const = None
            if mode == "compute":
                yt_const = cpool.tile([128, Bmax, F], gdt)
                nc.vector.memset(yt_const[:], 0.25)
            oh_const = None
            if mode == "nodve":
                oh_const = cpool.tile([128, WIN], gdt)
                nc.vector.memset(oh_const[:], 0.01)
            dump_sb = None
            if mode == "nomm":
                dump_sb = cpool.tile([128, N_WIN], f32)

            tb = 0
            col8 = 0
            for w in list(range(N_WIN)) * repeat:
                if w == 0:
                    tb = 0
                    col8 = 0
                bt = int(B_A[w] + B_B[w])
                yt = yt_const if mode == "compute" else ypool.tile([128, Bmax, F], gdt, tag="y")
                boff = 0
                for nblk, base_ap in ((int(B_A[w]), x_lo), (int(B_B[w]), x_hi)):
                    if nblk == 0 or mode == "compute":
                        continue
                    n = nblk * 128
                    nc.gpsimd.dma_gather(
                        yt[:, boff : boff + nblk, :],
                        base_ap,
                        idx_sb[:, col8 : col8 + nblk * 8],
                        n,
                        n,
                        F,
                        single_packet=SINGLE_PACKET,
                        queue_num=(col8 // 8) % NQ,
                    )
                    boff += nblk
                    col8 += nblk * 8

                if mode == "gather":
                    tb += bt
                    continue
                if mode == "nomm":
                    for j in range(bt):
                        oh = ohpool.tile([128, WIN], gdt, tag="oh")
                        nc.vector.tensor_scalar(
                            oh[:],
                            iota_sb[:],
                            srel_sb[:, tb + j : tb + j + 1],
                            normv_sb[:, tb + j : tb + j + 1],
                            mybir.AluOpType.is_equal,
                            mybir.AluOpType.mult,
                        )
                    nc.vector.tensor_copy(dump_sb[:, w : w + 1], yt[:, 0, 0:1])
                    tb += bt
                    continue
                ps_agg = psA.tile([128, WIN], f32, tag="psA")
                if ONEHOT_WIDE and mode != "nodve":
                    ohw = ohpool.tile([128, Bmax, WIN], gdt, tag="ohw")
                    nc.vector.tensor_tensor(
                        ohw[:, :bt, :],
                        iota_sb[:, :bt, :],
                        srel_sb[:, tb : tb + bt].to_broadcast([128, bt, WIN]),
                        mybir.AluOpType.is_equal,
                    )
                    if not NORM_FOLD:
                        nc.vector.tensor_tensor(
                            ohw[:, :bt, :],
                            ohw[:, :bt, :],
                            normv_sb[:, tb : tb + bt].to_broadcast([128, bt, WIN]),
                            mybir.AluOpType.mult,
                        )
                for j in range(bt):
                    if mode == "nodve":
                        oh = oh_const[:]
                    elif ONEHOT_WIDE:
                        oh = ohw[:, j, :]
                    else:
                        oht = ohpool.tile([128, WIN], gdt, tag="oh")
                        nc.vector.tensor_scalar(
                            oht[:],
                            iota_sb[:],
                            srel_sb[:, tb + j : tb + j + 1],
                            normv_sb[:, tb + j : tb + j + 1],
                            mybir.AluOpType.is_equal,
                            mybir.AluOpType.mult,
                        )
                        oh = oht[:]
                    nc.tensor.matmul(
                        ps_agg[:],
                        lhsT=yt[:, j, :],
                        rhs=oh,
                        start=(j == 0),
                        stop=(j == bt - 1),
                    )
                tb += bt

                aggT_sb = apool.tile([F, WIN], f32, tag="agg")
                nc.vector.tensor_copy(aggT_sb[:], ps_agg[:])

                ps_out = psO.tile([WIN, F], f32, tag="psO")
                bias_lhsT = (
                    invd_sb[0:1, w * WIN : (w + 1) * WIN] if NORM_FOLD else ones_sb[:]
                )
                nc.tensor.matmul(
                    ps_out[:], lhsT=bias_lhsT, rhs=brow_sb[:], start=True, stop=False
                )
                nc.tensor.matmul(
                    ps_out[:], lhsT=aggT_sb[:], rhs=wt_sb[:], start=False, stop=True
                )
                out_sb = opool.tile([WIN, F], f32, tag="out")
                nc.scalar.activation(
                    out_sb[:],
                    ps_out[:],
                    mybir.ActivationFunctionType.Relu,
                    scale=dinv_sb[:, w : w + 1] if NORM_FOLD else 1.0,
                )
                nc.sync.dma_start(out_d.ap()[w], out_sb[:])

    nc.compile()
    return nc


LAST_RESULTS = None


def kernel(x, edge_index, W, b, _trace=False):
    x = np.ascontiguousarray(np.asarray(x, dtype=np.float32))
    prep = _host_prep(x, edge_index, W, b)
    x_src = x * prep["dinv_full"][:, None] if NORM_FOLD else x
    x_dev = x_src.astype(ml_dtypes.bfloat16) if GATHER_BF16 else x_src

    nc = _build_program(prep["B_A"], prep["B_B"], prep["TB"], prep["Bmax"])

    in_maps = []
    for c in range(N_CORES):
        in_maps.append(
            {
                "x": x_dev,
                "idx16": prep["idx16"][c],
                "srel": prep["srel"][c],
                "normv": prep["normv"][c],
                "wt": prep["wt"],
                "brow": prep["brow"],
                "dinvc": prep["dinv_col"][c],
                "invd": prep["invd"][c],
                "iota": prep["iota"],
            }
        )

    global LAST_RESULTS
    res = run_bass_kernel_spmd(
        nc, in_maps, core_ids=list(range(N_CORES)), trace=_trace
    )
    LAST_RESULTS = res

    out = np.empty((N_NODES, F), dtype=np.float32)
    for c in range(N_CORES):
        o = res.results[c]["out"].reshape(N_WIN * WIN, F)
        out[c * NODES_PER_CORE : (c + 1) * NODES_PER_CORE] = o[:NODES_PER_CORE]
    if NORM_FOLD:
        z = prep["deg"] == 0
        if z.any():
            out[z] = np.maximum(np.asarray(b, dtype=np.float32), 0.0)[None, :]
    return out



# revision 2
# speedup vs baseline: 1.1187x; 1.1187x over previous
"""GCN layer (D^{-1/2} A D^{-1/2} X aggregated to src rows, then Linear+ReLU)
as a Bass/Tile kernel on 8 Trainium2 NeuronCores.

Strategy (v2):
  - Host: core c owns src rows [c*6250, (c+1)*6250) (49 windows of 128).
    x is pre-scaled by dinv (NORM_FOLD) and replicated to every core in bf16.
    Edges are bucketed per (core, window), split into lo/hi dst regions for
    int16 gather indices, and dst-sorted within each bucket.
  - Windows are processed in per-core order sorted by descending edge count so
    the static per-slot gather sizes (max over the 8 cores) hug each core's
    actual counts; the host unscrambles output rows afterwards.
  - Gather calls use exact num_idxs (16-aligned, row-0 padded) instead of
    128-block padding; one-hot sentinel (srel=300) zeroes pad lanes. The
    SWDGE descriptor-generation on GPSIMD is the kernel's critical path, so
    static descriptor count is minimized.
  - Device per window: dma_gather x[dst] rows (bf16) into SBUF; build the
    window's one-hot stack with one wide DVE is_equal; accumulate
    aggT[feat, src] via one-hot matmuls in PSUM; epilogue: out =
    relu(dinv_src * (agg @ W^T) + b) via bias-row matmul trick + Relu, then
    contiguous DMA store per slot.
"""

import ml_dtypes
import numpy as np

import concourse.bacc as bacc
import concourse.mybir as mybir
import concourse.tile as tile
from concourse.bass_utils import run_bass_kernel_spmd

N_NODES = 50000
N_EDGES = 800000
F = 128
N_CORES = 8
NODES_PER_CORE = N_NODES // N_CORES  # 6250
WIN = 128
N_WIN = -(-NODES_PER_CORE // WIN)  # 49
LO_BASE = 32768  # int16 index range per gather base
YBUFS = 3
OHBUFS = 4
PSABUFS = 2
PSOBUFS = 2
AGGBUFS = 3
OUTBUFS = 3
NQ = 4
SCRATCH = 65536
SENTINEL = 300.0


def _pack_idx16(idxs: np.ndarray) -> np.ndarray:
    """Pack an index vector (len multiple of 16) into the dma_gather idx tile
    layout: element i -> [i % 16, i // 16], replicated over 8 partition groups."""
    n = len(idxs)
    p16 = idxs.reshape(n // 16, 16).T.astype(np.int16)
    return np.tile(p16, (8, 1))


def _r16(n: int) -> int:
    return -(-n // 16) * 16


def _host_prep(x, edge_index):
    src = np.asarray(edge_index[0], dtype=np.int64)
    dst = np.asarray(edge_index[1], dtype=np.int64)
    deg = np.bincount(src, minlength=N_NODES).astype(np.float32)
    dinv = np.where(deg > 0, 1.0 / np.sqrt(deg), 0.0).astype(np.float32)

    order = np.argsort(src, kind="stable")
    src_s, dst_s = src[order], dst[order]

    core_of = src_s // NODES_PER_CORE
    wloc = (src_s % NODES_PER_CORE) // WIN
    is_hi = dst_s >= LO_BASE
    core_starts = np.searchsorted(core_of, np.arange(N_CORES + 1))

    # per (core, window): lo/hi edge index lists, dst-sorted
    buckets = {}
    tot = np.zeros((N_CORES, N_WIN), dtype=np.int64)
    for c in range(N_CORES):
        s, e = core_starts[c], core_starts[c + 1]
        wl = wloc[s:e]
        w_starts = np.searchsorted(wl, np.arange(N_WIN + 1)) + s
        for w in range(N_WIN):
            ws, we = w_starts[w], w_starts[w + 1]
            hi_m = is_hi[ws:we]
            lo_idx = np.arange(ws, we)[~hi_m]
            hi_idx = np.arange(ws, we)[hi_m]
            lo_idx = lo_idx[np.argsort(dst_s[lo_idx], kind="stable")]
            hi_idx = hi_idx[np.argsort(dst_s[hi_idx], kind="stable")]
            buckets[(c, w)] = (lo_idx, hi_idx)
            tot[c, w] = len(lo_idx) + len(hi_idx)

    # per-core window order: biggest windows first (aligns order statistics
    # across cores so the per-slot max is tight)
    worder = np.argsort(-tot, axis=1, kind="stable")  # [C, N_WIN]

    # static per-slot gather sizes (max over cores, 16-aligned)
    n_lo = np.zeros((N_CORES, N_WIN), dtype=np.int64)
    n_hi = np.zeros((N_CORES, N_WIN), dtype=np.int64)
    for c in range(N_CORES):
        for i in range(N_WIN):
            lo_idx, hi_idx = buckets[(c, worder[c, i])]
            n_lo[c, i] = len(lo_idx)
            n_hi[c, i] = len(hi_idx)
    NL = np.array([_r16(int(n_lo[:, i].max())) for i in range(N_WIN)])
    NH = np.array([_r16(int(n_hi[:, i].max())) for i in range(N_WIN)])
    BL = -(-NL // 128)
    BH = -(-NH // 128)
    BT = BL + BH
    TB = int(BT.sum())
    Bmax = int(BT.max())
    idx_cols = int((NL // 16 + NH // 16).sum())

    idx16 = np.zeros((N_CORES, 128, idx_cols), dtype=np.int16)
    srel = np.full((N_CORES, 128, TB), SENTINEL, dtype=np.float32)

    for c in range(N_CORES):
        col = 0
        tb = 0
        for i in range(N_WIN):
            w = worder[c, i]
            lo_idx, hi_idx = buckets[(c, w)]
            base_node = c * NODES_PER_CORE + w * WIN
            for edges, n_call, rebase in (
                (lo_idx, int(NL[i]), 0),
                (hi_idx, int(NH[i]), LO_BASE),
            ):
                if n_call == 0:
                    tb += 0
                    continue
                cnt = len(edges)
                dvals = np.zeros(n_call, dtype=np.int64)  # row-0 padding
                dvals[:cnt] = dst_s[edges] - rebase
                idx16[c, :, col : col + n_call // 16] = _pack_idx16(dvals)
                sv = np.full(-(-n_call // 128) * 128, SENTINEL, dtype=np.float32)
                sv[:cnt] = (src_s[edges] - base_node).astype(np.float32)
                nblk = -(-n_call // 128)
                srel[c, :, tb : tb + nblk] = sv.reshape(nblk, 128).T
                col += n_call // 16
                tb += nblk

    srel = srel.astype(ml_dtypes.bfloat16)
    iota = np.broadcast_to(
        np.arange(WIN, dtype=np.float32).astype(ml_dtypes.bfloat16), (128, Bmax, WIN)
    ).copy()

    # per-core, slot-ordered dinv columns (epilogue scale) and inverse (bias)
    dinv_col = np.zeros((N_CORES, WIN, N_WIN), dtype=np.float32)
    invd = np.zeros((N_CORES, 1, N_WIN * WIN), dtype=np.float32)
    for c in range(N_CORES):
        dv_full = np.zeros(N_WIN * WIN, dtype=np.float32)
        dv_full[:NODES_PER_CORE] = dinv[c * NODES_PER_CORE : (c + 1) * NODES_PER_CORE]
        dv_slot = np.zeros(N_WIN * WIN, dtype=np.float32)
        for i in range(N_WIN):
            w = worder[c, i]
            dv_slot[i * WIN : (i + 1) * WIN] = dv_full[w * WIN : (w + 1) * WIN]
        dinv_col[c] = dv_slot.reshape(N_WIN, WIN).T
        iv = np.zeros_like(dv_slot)
        nz = dv_slot > 0
        iv[nz] = 1.0 / dv_slot[nz]
        invd[c, 0] = iv

    return {
        "deg": deg,
        "dinv_full": dinv,
        "worder": worder,
        "dinv_col": dinv_col,
        "invd": invd,
        "NL": NL,
        "NH": NH,
        "BL": BL,
        "BH": BH,
        "TB": TB,
        "Bmax": Bmax,
        "idx_cols": idx_cols,
        "idx16": idx16,
        "srel": srel,
        "iota": iota,
    }


def _build_program(NL, NH, BL, BH, TB, Bmax, idx_cols):
    f32 = mybir.dt.float32
    bf16 = mybir.dt.bfloat16
    nc = bacc.Bacc(
        "TRN2",
        target_bir_lowering=False,
        debug=False,
        num_devices=1,
        num_swdge_queues=NQ,
        dynamic_dma_scratch_size=SCRATCH,
    )

    # idx cols for the first two slots load first so gathers start immediately
    head_cols = int(NL[0] // 16 + NH[0] // 16 + NL[1] // 16 + NH[1] // 16)
    tail_cols = idx_cols - head_cols

    x_d = nc.dram_tensor("x", [N_NODES, F], bf16, kind="ExternalInput")
    idxh_d = nc.dram_tensor("idxh", [128, head_cols], mybir.dt.int16, kind="ExternalInput")
    idxt_d = nc.dram_tensor("idxt", [128, max(tail_cols, 16)], mybir.dt.int16, kind="ExternalInput")
    srel_d = nc.dram_tensor("srel", [128, TB], bf16, kind="ExternalInput")
    wt_d = nc.dram_tensor("wt", [F, F], f32, kind="ExternalInput")
    brow_d = nc.dram_tensor("brow", [1, F], f32, kind="ExternalInput")
    dinv_d = nc.dram_tensor("dinvc", [WIN, N_WIN], f32, kind="ExternalInput")
    invd_d = nc.dram_tensor("invd", [1, N_WIN * WIN], f32, kind="ExternalInput")
    iota_d = nc.dram_tensor("iota", [128, Bmax, WIN], bf16, kind="ExternalInput")
    out_d = nc.dram_tensor("out", [N_WIN, WIN, F], f32, kind="ExternalOutput")

    x_lo = x_d.ap()[0:LO_BASE, :]
    x_hi = x_d.ap()[LO_BASE:N_NODES, :]

    with tile.TileContext(nc) as tc:
        with (
            tc.tile_pool(name="const", bufs=1) as cpool,
            tc.tile_pool(name="y", bufs=YBUFS) as ypool,
            tc.tile_pool(name="oh", bufs=OHBUFS) as ohpool,
            tc.tile_pool(name="agg", bufs=AGGBUFS) as apool,
            tc.tile_pool(name="outp", bufs=OUTBUFS) as opool,
            tc.tile_pool(name="psA", bufs=PSABUFS, space="PSUM") as psA,
            tc.tile_pool(name="psO", bufs=PSOBUFS, space="PSUM") as psO,
        ):
            idxh_sb = cpool.tile([128, head_cols], mybir.dt.int16)
            nc.sync.dma_start(idxh_sb[:], idxh_d.ap())
            idxt_sb = cpool.tile([128, max(tail_cols, 16)], mybir.dt.int16)
            nc.sync.dma_start(idxt_sb[:], idxt_d.ap())
            srel_sb = cpool.tile([128, TB], bf16)
            nc.sync.dma_start(srel_sb[:], srel_d.ap())
            wt_sb = cpool.tile([F, F], f32)
            nc.sync.dma_start(wt_sb[:], wt_d.ap())
            brow_sb = cpool.tile([1, F], f32)
            nc.sync.dma_start(brow_sb[:], brow_d.ap())
            dinv_sb = cpool.tile([WIN, N_WIN], f32)
            nc.sync.dma_start(dinv_sb[:], dinv_d.ap())
            invd_sb = cpool.tile([1, N_WIN * WIN], f32)
            nc.sync.dma_start(invd_sb[:], invd_d.ap())
            iota_sb = cpool.tile([128, Bmax, WIN], bf16)
            nc.sync.dma_start(iota_sb[:], iota_d.ap())

            # pre-zero the gather buffers: exact num_idxs leaves tail lanes of
            # the last block stale; only the first YBUFS uses see uninit SBUF.
            ybufs = []
            for _ in range(YBUFS):
                yt0 = ypool.tile([128, Bmax, F], bf16, tag="y")
                nc.vector.memset(yt0[:], 0.0)
                ybufs.append(yt0)

            tb = 0
            colh = 0
            colt = 0
            qn = 0
            for i in range(N_WIN):
                bl, bh = int(BL[i]), int(BH[i])
                bt = bl + bh
                yt = ybufs[i] if i < YBUFS else ypool.tile([128, Bmax, F], bf16, tag="y")
                boff = 0
                for n_call, base_ap in ((int(NL[i]), x_lo), (int(NH[i]), x_hi)):
                    if n_call == 0:
                        continue
                    nblk = -(-n_call // 128)
                    ncols = n_call // 16
                    if i < 2:
                        idx_slice = idxh_sb[:, colh : colh + ncols]
                        colh += ncols
                    else:
                        idx_slice = idxt_sb[:, colt : colt + ncols]
                        colt += ncols
                    nc.gpsimd.dma_gather(
                        yt[:, boff : boff + nblk, :],
                        base_ap,
                        idx_slice,
                        n_call,
                        n_call,
                        F,
                        single_packet=False,
                        queue_num=qn % NQ,
                    )
                    qn += 1
                    boff += nblk

                ps_agg = psA.tile([128, WIN], f32, tag="psA")
                ohw = ohpool.tile([128, Bmax, WIN], bf16, tag="ohw")
                nc.vector.tensor_tensor(
                    ohw[:, :bt, :],
                    iota_sb[:, :bt, :],
                    srel_sb[:, tb : tb + bt].to_broadcast([128, bt, WIN]),
                    mybir.AluOpType.is_equal,
                )
                for j in range(bt):
                    nc.tensor.matmul(
                        ps_agg[:],
                        lhsT=yt[:, j, :],
                        rhs=ohw[:, j, :],
                        start=(j == 0),
                        stop=(j == bt - 1),
                    )
                tb += bt

                aggT_sb = apool.tile([F, WIN], f32, tag="agg")
                nc.vector.tensor_copy(aggT_sb[:], ps_agg[:])

                ps_out = psO.tile([WIN, F], f32, tag="psO")
                nc.tensor.matmul(
                    ps_out[:],
                    lhsT=invd_sb[0:1, i * WIN : (i + 1) * WIN],
                    rhs=brow_sb[:],
                    start=True,
                    stop=False,
                )
                nc.tensor.matmul(
                    ps_out[:], lhsT=aggT_sb[:], rhs=wt_sb[:], start=False, stop=True
                )
                out_sb = opool.tile([WIN, F], f32, tag="out")
                nc.scalar.activation(
                    out_sb[:],
                    ps_out[:],
                    mybir.ActivationFunctionType.Relu,
                    scale=dinv_sb[:, i : i + 1],
                )
                nc.sync.dma_start(out_d.ap()[i], out_sb[:])

    nc.compile()
    return nc


LAST_RESULTS = None


def kernel(x, edge_index, W, b, _trace=False):
    x = np.ascontiguousarray(np.asarray(x, dtype=np.float32))
    W = np.asarray(W, dtype=np.float32)
    b = np.asarray(b, dtype=np.float32)
    prep = _host_prep(x, edge_index)
    x_dev = (x * prep["dinv_full"][:, None]).astype(ml_dtypes.bfloat16)

    nc = _build_program(
        prep["NL"], prep["NH"], prep["BL"], prep["BH"], prep["TB"], prep["Bmax"],
        prep["idx_cols"],
    )

    wt = np.ascontiguousarray(W.T)  # [in, out]
    brow = b.reshape(1, F)
    head_cols = int(
        prep["NL"][0] // 16 + prep["NH"][0] // 16
        + prep["NL"][1] // 16 + prep["NH"][1] // 16
    )
    tail_cols = prep["idx_cols"] - head_cols

    in_maps = []
    for c in range(N_CORES):
        idxh = prep["idx16"][c][:, :head_cols]
        idxt = prep["idx16"][c][:, head_cols:]
        if tail_cols < 16:
            idxt = np.zeros((128, 16), dtype=np.int16)
        in_maps.append(
            {
                "x": x_dev,
                "idxh": np.ascontiguousarray(idxh),
                "idxt": np.ascontiguousarray(idxt),
                "srel": prep["srel"][c],
                "wt": wt,
                "brow": brow,
                "dinvc": prep["dinv_col"][c],
                "invd": prep["invd"][c],
                "iota": prep["iota"],
            }
        )

    global LAST_RESULTS
    res = run_bass_kernel_spmd(
        nc, in_maps, core_ids=list(range(N_CORES)), trace=_trace
    )
    LAST_RESULTS = res

    out = np.empty((N_NODES, F), dtype=np.float32)
    for c in range(N_CORES):
        o = res.results[c]["out"]  # [N_WIN, WIN, F] in slot order
        base = c * NODES_PER_CORE
        for i in range(N_WIN):
            w = int(prep["worder"][c, i])
            r0 = w * WIN
            r1 = min(r0 + WIN, NODES_PER_CORE)
            out[base + r0 : base + r1] = o[i, : r1 - r0]
    z = prep["deg"] == 0
    if z.any():
        out[z] = np.maximum(b, 0.0)[None, :]
    return out


# revision 8
# speedup vs baseline: 1.2399x; 1.1083x over previous
"""GCN layer (D^{-1/2} A D^{-1/2} X aggregated to src rows, then Linear+ReLU)
as a Bass/Tile kernel on 8 Trainium2 NeuronCores.

Strategy (v2):
  - Host: core c owns src rows [c*6250, (c+1)*6250) (49 windows of 128).
    x is pre-scaled by dinv (NORM_FOLD) and replicated to every core in bf16.
    Edges are bucketed per (core, window), split into lo/hi dst regions for
    int16 gather indices, and dst-sorted within each bucket.
  - Windows are processed in per-core order sorted by descending edge count so
    the static per-slot gather sizes (max over the 8 cores) hug each core's
    actual counts; the host unscrambles output rows afterwards.
  - Gather calls use exact num_idxs (16-aligned, row-0 padded) instead of
    128-block padding; one-hot sentinel (srel=300) zeroes pad lanes. The
    SWDGE descriptor-generation on GPSIMD is the kernel's critical path, so
    static descriptor count is minimized.
  - Device per window: dma_gather x[dst] rows (bf16) into SBUF; build the
    window's one-hot stack with one wide DVE is_equal; accumulate
    aggT[feat, src] via one-hot matmuls in PSUM; epilogue: out =
    relu(dinv_src * (agg @ W^T) + b) via bias-row matmul trick + Relu, then
    contiguous DMA store per slot.
"""

import ml_dtypes
import numpy as np

import concourse.bacc as bacc
import concourse.mybir as mybir
import concourse.tile as tile
from concourse.bass_utils import run_bass_kernel_spmd

N_NODES = 50000
N_EDGES = 800000
F = 128
N_CORES = 8
NODES_PER_CORE = N_NODES // N_CORES  # 6250
WIN = 128
N_WIN = -(-NODES_PER_CORE // WIN)  # 49
LO_BASE = 32768  # region A covers rows [0, 32768)
HI_BASE = N_NODES - 32768  # region B covers rows [17232, 50000)
YBUFS = 3
OHBUFS = 4
PSABUFS = 2
PSOBUFS = 2
AGGBUFS = 3
OUTBUFS = 3
NQ = 4
SCRATCH = 65536
SENTINEL = 300.0


def _pack_idx16(idxs: np.ndarray) -> np.ndarray:
    """Pack an index vector (len multiple of 16) into the dma_gather idx tile
    layout: element i -> [i % 16, i // 16], replicated over 8 partition groups."""
    n = len(idxs)
    p16 = idxs.reshape(n // 16, 16).T.astype(np.int16)
    return np.tile(p16, (8, 1))


def _r16(n: int) -> int:
    return -(-n // 16) * 16


def _host_prep(x, edge_index):
    src = np.asarray(edge_index[0], dtype=np.int64)
    dst = np.asarray(edge_index[1], dtype=np.int64)
    deg = np.bincount(src, minlength=N_NODES).astype(np.float32)
    dinv = np.where(deg > 0, 1.0 / np.sqrt(deg), 0.0).astype(np.float32)

    order = np.argsort(src, kind="stable")
    src_s, dst_s = src[order], dst[order]

    core_of = src_s // NODES_PER_CORE
    wloc = (src_s % NODES_PER_CORE) // WIN
    core_starts = np.searchsorted(core_of, np.arange(N_CORES + 1))

    # per (core, window): dst-sorted edge list, split into a balanced lo/hi
    # pair (region A = [0, LO_BASE), region B = [HI_BASE, N); dsts in the
    # overlap go to whichever side balances the two gather calls)
    buckets = {}
    tot = np.zeros((N_CORES, N_WIN), dtype=np.int64)
    for c in range(N_CORES):
        s, e = core_starts[c], core_starts[c + 1]
        wl = wloc[s:e]
        w_starts = np.searchsorted(wl, np.arange(N_WIN + 1)) + s
        for w in range(N_WIN):
            ws, we = w_starts[w], w_starts[w + 1]
            eidx = np.arange(ws, we)
            eidx = eidx[np.argsort(dst_s[eidx], kind="stable")]
            dd = dst_s[eidx]
            n = len(eidx)
            n_min = int(np.searchsorted(dd, HI_BASE))  # must go to A
            n_max = int(np.searchsorted(dd, LO_BASE))  # can go to A
            n_a = min(max((n + 1) // 2, n_min), n_max)
            buckets[(c, w)] = (eidx[:n_a], eidx[n_a:])
            tot[c, w] = n

    # per-core window order: biggest windows first (aligns order statistics
    # across cores so the per-slot max is tight)
    worder = np.argsort(-tot, axis=1, kind="stable")  # [C, N_WIN]

    # static per-slot gather sizes (max over cores, 16-aligned)
    n_lo = np.zeros((N_CORES, N_WIN), dtype=np.int64)
    n_hi = np.zeros((N_CORES, N_WIN), dtype=np.int64)
    for c in range(N_CORES):
        for i in range(N_WIN):
            lo_idx, hi_idx = buckets[(c, worder[c, i])]
            n_lo[c, i] = len(lo_idx)
            n_hi[c, i] = len(hi_idx)
    NL = np.array([_r16(int(n_lo[:, i].max())) for i in range(N_WIN)])
    NH = np.array([_r16(int(n_hi[:, i].max())) for i in range(N_WIN)])
    BL = -(-NL // 128)
    BH = -(-NH // 128)
    BT = BL + BH
    TB = int(BT.sum())
    Bmax = int(BT.max())
    idx_cols = int((NL // 16 + NH // 16).sum())

    idx16 = np.zeros((N_CORES, 128, idx_cols), dtype=np.int16)
    srel = np.full((N_CORES, 128, TB), SENTINEL, dtype=np.float32)

    for c in range(N_CORES):
        col = 0
        tb = 0
        for i in range(N_WIN):
            w = worder[c, i]
            lo_idx, hi_idx = buckets[(c, w)]
            base_node = c * NODES_PER_CORE + w * WIN
            for edges, n_call, rebase in (
                (lo_idx, int(NL[i]), 0),
                (hi_idx, int(NH[i]), HI_BASE),
            ):
                if n_call == 0:
                    tb += 0
                    continue
                cnt = len(edges)
                dvals = np.zeros(n_call, dtype=np.int64)  # row-0 padding
                dvals[:cnt] = dst_s[edges] - rebase
                idx16[c, :, col : col + n_call // 16] = _pack_idx16(dvals)
                sv = np.full(-(-n_call // 128) * 128, SENTINEL, dtype=np.float32)
                sv[:cnt] = (src_s[edges] - base_node).astype(np.float32)
                nblk = -(-n_call // 128)
                srel[c, :, tb : tb + nblk] = sv.reshape(nblk, 128).T
                col += n_call // 16
                tb += nblk

    srel = srel.astype(ml_dtypes.bfloat16)
    iota = np.broadcast_to(
        np.arange(WIN, dtype=np.float32).astype(ml_dtypes.bfloat16), (128, Bmax, WIN)
    ).copy()

    # per-core, slot-ordered dinv columns (epilogue scale) and inverse (bias)
    dinv_col = np.zeros((N_CORES, WIN, N_WIN), dtype=np.float32)
    invd = np.zeros((N_CORES, 1, N_WIN * WIN), dtype=np.float32)
    for c in range(N_CORES):
        dv_full = np.zeros(N_WIN * WIN, dtype=np.float32)
        dv_full[:NODES_PER_CORE] = dinv[c * NODES_PER_CORE : (c + 1) * NODES_PER_CORE]
        dv_slot = np.zeros(N_WIN * WIN, dtype=np.float32)
        for i in range(N_WIN):
            w = worder[c, i]
            dv_slot[i * WIN : (i + 1) * WIN] = dv_full[w * WIN : (w + 1) * WIN]
        dinv_col[c] = dv_slot.reshape(N_WIN, WIN).T
        iv = np.zeros_like(dv_slot)
        nz = dv_slot > 0
        iv[nz] = 1.0 / dv_slot[nz]
        invd[c, 0] = iv

    return {
        "deg": deg,
        "dinv_full": dinv,
        "worder": worder,
        "dinv_col": dinv_col,
        "invd": invd,
        "NL": NL,
        "NH": NH,
        "BL": BL,
        "BH": BH,
        "TB": TB,
        "Bmax": Bmax,
        "idx_cols": idx_cols,
        "idx16": idx16,
        "srel": srel,
        "iota": iota,
    }


def _build_program(NL, NH, BL, BH, TB, Bmax, idx_cols):
    f32 = mybir.dt.float32
    bf16 = mybir.dt.bfloat16
    nc = bacc.Bacc(
        "TRN2",
        target_bir_lowering=False,
        debug=False,
        num_devices=1,
        num_swdge_queues=NQ,
        dynamic_dma_scratch_size=SCRATCH,
    )

    # idx cols for the first two slots load first so gathers start immediately
    head_cols = int(NL[0] // 16 + NH[0] // 16 + NL[1] // 16 + NH[1] // 16)
    tail_cols = idx_cols - head_cols

    x_d = nc.dram_tensor("x", [N_NODES, F], bf16, kind="ExternalInput")
    idxh_d = nc.dram_tensor("idxh", [128, head_cols], mybir.dt.int16, kind="ExternalInput")
    idxt_d = nc.dram_tensor("idxt", [128, max(tail_cols, 16)], mybir.dt.int16, kind="ExternalInput")
    srel_d = nc.dram_tensor("srel", [128, TB], bf16, kind="ExternalInput")
    wt_d = nc.dram_tensor("wt", [F, F], f32, kind="ExternalInput")
    brow_d = nc.dram_tensor("brow", [1, F], f32, kind="ExternalInput")
    dinv_d = nc.dram_tensor("dinvc", [WIN, N_WIN], f32, kind="ExternalInput")
    invd_d = nc.dram_tensor("invd", [1, N_WIN * WIN], f32, kind="ExternalInput")
    iota_d = nc.dram_tensor("iota", [128, Bmax, WIN], bf16, kind="ExternalInput")
    out_d = nc.dram_tensor("out", [N_WIN, WIN, F], f32, kind="ExternalOutput")

    x_lo = x_d.ap()[0:LO_BASE, :]
    x_hi = x_d.ap()[HI_BASE:N_NODES, :]

    with tile.TileContext(nc) as tc:
        with (
            tc.tile_pool(name="const", bufs=1) as cpool,
            tc.tile_pool(name="y", bufs=YBUFS) as ypool,
            tc.tile_pool(name="oh", bufs=OHBUFS) as ohpool,
            tc.tile_pool(name="agg", bufs=AGGBUFS) as apool,
            tc.tile_pool(name="outp", bufs=OUTBUFS) as opool,
            tc.tile_pool(name="psA", bufs=PSABUFS, space="PSUM") as psA,
            tc.tile_pool(name="psO", bufs=PSOBUFS, space="PSUM") as psO,
        ):
            idxh_sb = cpool.tile([128, head_cols], mybir.dt.int16)
            nc.sync.dma_start(idxh_sb[:], idxh_d.ap())
            idxt_sb = cpool.tile([128, max(tail_cols, 16)], mybir.dt.int16)
            nc.sync.dma_start(idxt_sb[:], idxt_d.ap())
            srel_sb = cpool.tile([128, TB], bf16)
            nc.sync.dma_start(srel_sb[:], srel_d.ap())
            wt_sb = cpool.tile([F, F], f32)
            nc.sync.dma_start(wt_sb[:], wt_d.ap())
            brow_sb = cpool.tile([1, F], f32)
            nc.sync.dma_start(brow_sb[:], brow_d.ap())
            dinv_sb = cpool.tile([WIN, N_WIN], f32)
            nc.sync.dma_start(dinv_sb[:], dinv_d.ap())
            invd_sb = cpool.tile([1, N_WIN * WIN], f32)
            nc.sync.dma_start(invd_sb[:], invd_d.ap())
            iota_sb = cpool.tile([128, Bmax, WIN], bf16)
            nc.sync.dma_start(iota_sb[:], iota_d.ap())

            # pre-zero the gather buffers: exact num_idxs leaves tail lanes of
            # the last block stale; only the first YBUFS uses see uninit SBUF.
            ybufs = []
            for _ in range(YBUFS):
                yt0 = ypool.tile([128, Bmax, F], bf16, tag="y")
                nc.vector.memset(yt0[:], 0.0)
                ybufs.append(yt0)

            tb = 0
            colh = 0
            colt = 0
            qload = [0] * NQ
            for i in range(N_WIN):
                bl, bh = int(BL[i]), int(BH[i])
                bt = bl + bh
                yt = ybufs[i] if i < YBUFS else ypool.tile([128, Bmax, F], bf16, tag="y")
                boff = 0
                for n_call, base_ap in ((int(NL[i]), x_lo), (int(NH[i]), x_hi)):
                    if n_call == 0:
                        continue
                    nblk = -(-n_call // 128)
                    ncols = n_call // 16
                    if i < 2:
                        idx_slice = idxh_sb[:, colh : colh + ncols]
                        colh += ncols
                    else:
                        idx_slice = idxt_sb[:, colt : colt + ncols]
                        colt += ncols
                    qn = min(range(NQ), key=lambda q: qload[q])
                    qload[qn] += n_call
                    nc.gpsimd.dma_gather(
                        yt[:, boff : boff + nblk, :],
                        base_ap,
                        idx_slice,
                        n_call,
                        n_call,
                        F,
                        single_packet=False,
                        queue_num=qn,
                    )
                    boff += nblk

                ps_agg = psA.tile([128, WIN], f32, tag="psA")
                ohw = ohpool.tile([128, Bmax, WIN], bf16, tag="ohw")
                nc.vector.tensor_tensor(
                    ohw[:, :bt, :],
                    iota_sb[:, :bt, :],
                    srel_sb[:, tb : tb + bt].to_broadcast([128, bt, WIN]),
                    mybir.AluOpType.is_equal,
                )
                for j in range(bt):
                    nc.tensor.matmul(
                        ps_agg[:],
                        lhsT=yt[:, j, :],
                        rhs=ohw[:, j, :],
                        start=(j == 0),
                        stop=(j == bt - 1),
                    )
                tb += bt

                aggT_sb = apool.tile([F, WIN], f32, tag="agg")
                nc.vector.tensor_copy(aggT_sb[:], ps_agg[:])

                ps_out = psO.tile([WIN, F], f32, tag="psO")
                nc.tensor.matmul(
                    ps_out[:],
                    lhsT=invd_sb[0:1, i * WIN : (i + 1) * WIN],
                    rhs=brow_sb[:],
                    start=True,
                    stop=False,
                )
                nc.tensor.matmul(
                    ps_out[:], lhsT=aggT_sb[:], rhs=wt_sb[:], start=False, stop=True
                )
                out_sb = opool.tile([WIN, F], f32, tag="out")
                nc.scalar.activation(
                    out_sb[:],
                    ps_out[:],
                    mybir.ActivationFunctionType.Relu,
                    scale=dinv_sb[:, i : i + 1],
                )
                nc.sync.dma_start(out_d.ap()[i], out_sb[:])

    nc.compile()
    return nc


LAST_RESULTS = None


def kernel(x, edge_index, W, b, _trace=False):
    x = np.ascontiguousarray(np.asarray(x, dtype=np.float32))
    W = np.asarray(W, dtype=np.float32)
    b = np.asarray(b, dtype=np.float32)
    prep = _host_prep(x, edge_index)
    x_dev = (x * prep["dinv_full"][:, None]).astype(ml_dtypes.bfloat16)

    nc = _build_program(
        prep["NL"], prep["NH"], prep["BL"], prep["BH"], prep["TB"], prep["Bmax"],
        prep["idx_cols"],
    )

    wt = np.ascontiguousarray(W.T)  # [in, out]
    brow = b.reshape(1, F)
    head_cols = int(
        prep["NL"][0] // 16 + prep["NH"][0] // 16
        + prep["NL"][1] // 16 + prep["NH"][1] // 16
    )
    tail_cols = prep["idx_cols"] - head_cols

    in_maps = []
    for c in range(N_CORES):
        idxh = prep["idx16"][c][:, :head_cols]
        idxt = prep["idx16"][c][:, head_cols:]
        if tail_cols < 16:
            idxt = np.zeros((128, 16), dtype=np.int16)
        in_maps.append(
            {
                "x": x_dev,
                "idxh": np.ascontiguousarray(idxh),
                "idxt": np.ascontiguousarray(idxt),
                "srel": prep["srel"][c],
                "wt": wt,
                "brow": brow,
                "dinvc": prep["dinv_col"][c],
                "invd": prep["invd"][c],
                "iota": prep["iota"],
            }
        )

    global LAST_RESULTS
    res = run_bass_kernel_spmd(
        nc, in_maps, core_ids=list(range(N_CORES)), trace=_trace
    )
    LAST_RESULTS = res

    out = np.empty((N_NODES, F), dtype=np.float32)
    for c in range(N_CORES):
        o = res.results[c]["out"]  # [N_WIN, WIN, F] in slot order
        base = c * NODES_PER_CORE
        for i in range(N_WIN):
            w = int(prep["worder"][c, i])
            r0 = w * WIN
            r1 = min(r0 + WIN, NODES_PER_CORE)
            out[base + r0 : base + r1] = o[i, : r1 - r0]
    z = prep["deg"] == 0
    if z.any():
        out[z] = np.maximum(b, 0.0)[None, :]
    return out
